# revision 2
# baseline (speedup 1.0000x reference)
"""CACE GNN message-passing kernel for 8 trn2 NeuronCores.

Sharding: node-parallel. Edges are sorted by receiver and assigned to the core
owning the receiver node range (625 nodes/core). Each core:
  1. computes per-edge radial[8] / angular[20] / encoded[9] factors,
  2. scatter-adds rank-1 edge tensors into node buckets A[n,r,m,c] with ONE
     matmul per edge-chunk (lhsT = onehot(node)*radial [128e,112=(14n,8r)],
     rhs = ang x enc [128e,180=(20m,9c)]),
  3. evaluates the nu=2..4 symmetrization via a closed-form tensor-contraction
     plan (u=Sa, z=T:S, P=T:a, M=T:T, S2=S.S, ...) on DVE/GPSIMD/ACT.
No cross-core communication needed (pure node sharding).
"""
import math
import functools
import numpy as np

# ---------------- problem constants (hardcoded; must match reference) -------
N_NODES, N_EDGES = 5000, 50000
N_RBF, MAX_L = 8, 3
CUTOFF = 5.5
EPS = 1e-9
ZS = [1, 6, 7, 8]
N_CORES = 8
PER = N_NODES // N_CORES          # 625 nodes per core
NT = 14                           # nodes per chunk-tile
N_CH = 56                         # chunk-tiles per core (padded)
P = 128                           # edges per chunk (partitions)
NQ = NT * N_RBF                   # 112 = lhsT free
NM = 20                           # angular monomials
NC9 = 9                           # encoded channels
NF = 11                           # output features
SQ2C = math.sqrt(2.0 / CUTOFF)


def _lxlylz_list(max_l=3):
    lst = []
    for l in range(max_l + 1):
        for lx in range(l, -1, -1):
            for ly in range(l - lx, -1, -1):
                lst.append((lx, ly, l - lx - ly))
    return lst


LXLYLZ = _lxlylz_list()
IDX = {v: i for i, v in enumerate(LXLYLZ)}


def _e(i):
    v = [0, 0, 0]
    v[i] += 1
    return tuple(v)


def _vadd(*vs):
    o = [0, 0, 0]
    for v in vs:
        o = [o[k] + v[k] for k in range(3)]
    return tuple(o)


A_ = [IDX[_e(a)] for a in range(3)]
S_ = {(a, b): IDX[_vadd(_e(a), _e(b))] for a in range(3) for b in range(3)}
T_ = {(a, b, c): IDX[_vadd(_e(a), _e(b), _e(c))]
      for a in range(3) for b in range(3) for c in range(3)}
SYM2 = [(0, 0), (0, 1), (0, 2), (1, 1), (1, 2), (2, 2)]
W2 = {p: (1.0 if p[0] == p[1] else 2.0) for p in SYM2}
SYM3 = sorted({tuple(sorted(k)) for k in T_})


def _w3(t):
    cnt = {}
    for x in t:
        cnt[x] = cnt.get(x, 0) + 1
    r = math.factorial(3)
    for v in cnt.values():
        r //= math.factorial(v)
    return float(r)


# ---------------- symmetrization plan --------------------------------------
class _Plan:
    def __init__(self):
        self.ops = []
        self.nt = 0

    def t(self):
        self.nt += 1
        return ('t', self.nt - 1)

    def mul(self, a, b, eng='v'):
        d = self.t()
        self.ops.append((eng, 'mul', d, a, b))
        return d

    def add(self, a, b, eng='v'):
        d = self.t()
        self.ops.append((eng, 'add', d, a, b))
        return d

    def sq(self, a):
        d = self.t()
        self.ops.append(('s', 'sq', d, a))
        return d

    def wmul(self, a, w):
        d = self.t()
        self.ops.append(('s', 'wmul', d, a, float(w)))
        return d

    def dot(self, pairs, eng='v'):
        # sum w*Pa*Pb, grouping weights to minimize wmuls
        by_w = {}
        for (a, b, w) in pairs:
            by_w.setdefault(float(w), []).append((a, b))
        total = None
        for w, lst in sorted(by_w.items()):
            acc = None
            for (a, b) in lst:
                pr = self.mul(a, b, eng=eng)
                acc = pr if acc is None else self.add(acc, pr, eng=eng)
            if w != 1.0:
                acc = self.wmul(acc, w)
            total = acc if total is None else self.add(total, acc, eng=eng)
        return total


def build_plan():
    p = _Plan()
    A = lambda m: ('A', m)
    def SQ(m):
        return ('Q', m)

    # nu2: grouped-weight sums of squares
    def wsq_sum(items, eng='v'):
        by_w = {}
        for (m, w) in items:
            by_w.setdefault(float(w), []).append(m)
        total = None
        for w, ms in sorted(by_w.items()):
            acc = None
            for m in ms:
                acc = SQ(m) if acc is None else p.add(acc, SQ(m), eng=eng)
            if w != 1.0:
                acc = p.wmul(acc, w)
            total = acc if total is None else p.add(total, acc, eng=eng)
        return total

    nu2_1 = wsq_sum([(A_[a], 1.0) for a in range(3)])
    nu2_2 = wsq_sum([(S_[ab], W2[ab]) for ab in SYM2])
    nu2_3 = wsq_sum([(T_[t3], _w3(t3)) for t3 in SYM3])
    u = [p.dot([(A(S_[(a, b)]), A(A_[b]), 1.0) for b in range(3)]) for a in range(3)]
    z = [p.dot([(A(T_[tuple(sorted((a, b, c)))]), A(S_[(a, b)]), W2[(a, b)])
                for (a, b) in SYM2]) for c in range(3)]
    P2 = {bc: p.dot([(A(A_[a]), A(T_[tuple(sorted((a,) + bc))]), 1.0)
                     for a in range(3)]) for bc in SYM2}
    S2 = {ab: p.dot([(A(S_[(ab[0], k)]), A(S_[(k, ab[1])]), 1.0)
                     for k in range(3)], eng='g') for ab in SYM2}
    M = {cd: p.dot([(A(T_[tuple(sorted((a, b, cd[0])))]),
                     A(T_[tuple(sorted((a, b, cd[1])))]), W2[(a, b)])
                    for (a, b) in SYM2], eng='g') for cd in SYM2}
    trS3 = p.dot([(S2[ab], A(S_[ab]), W2[ab]) for ab in SYM2], eng='g')
    nu3_2 = p.dot([(M[cd], A(S_[cd]), W2[cd]) for cd in SYM2], eng='g')
    nu4_1 = p.dot([(u[a], u[a], 1.0) for a in range(3)])
    nu4_2 = p.dot([(u[a], z[a], 1.0) for a in range(3)])
    nu4_3 = p.dot([(P2[bc], P2[bc], W2[bc]) for bc in SYM2])
    nu4_5 = p.dot([(z[a], z[a], 1.0) for a in range(3)])
    feats = [nu2_1, nu2_2, nu2_3, trS3, nu3_2, nu4_1, nu4_2, nu4_3, nu4_2, nu4_5]
    for f, src in enumerate(feats):
        p.ops.append(('s', 'copy', ('F', f + 1), src))
    p.ops.append(('s', 'copy', ('F', 0), ('A', 0)))
    return p


def run_plan_numpy(plan, Ap):
    env = {('A', m): Ap[:, m] for m in range(20)}
    env.update({('Q', m): Ap[:, m] ** 2 for m in range(20)})
    F = np.zeros((Ap.shape[0], 11), Ap.dtype)
    for op in plan.ops:
        kind = op[1]
        dst = op[2]
        if kind == 'mul':
            v = env[op[3]] * env[op[4]]
        elif kind == 'add':
            v = env[op[3]] + env[op[4]]
        elif kind == 'sq':
            v = env[op[3]] ** 2
        elif kind == 'wmul':
            v = env[op[3]] * op[4]
        elif kind == 'copy':
            v = env[op[3]]
        if dst[0] == 'F':
            F[:, dst[1]] = v
        else:
            env[dst] = v
    return F


def _alloc_slots(plan):
    """linear-scan register allocation for scratch planes -> slot ids"""
    last_use = {}
    for i, op in enumerate(plan.ops):
        for x in op[3:]:
            if isinstance(x, tuple) and x[0] == 't':
                last_use[x] = i
    slot_of = {}
    free = []
    n_slots = 0
    for i, op in enumerate(plan.ops):
        dst = op[2]
        if dst[0] == 't':
            if free:
                slot_of[dst] = free.pop()
            else:
                slot_of[dst] = n_slots
                n_slots += 1
        for x in op[3:]:
            if isinstance(x, tuple) and x[0] == 't' and last_use.get(x) == i:
                free.append(slot_of[x])
    return slot_of, n_slots


# ---------------- device kernel build --------------------------------------
@functools.lru_cache(maxsize=2)
def _build_nc(debug=False):
    import concourse.bass as bass
    import concourse.bacc as bacc
    import concourse.mybir as mybir
    from concourse.tile import TileContext

    dt = mybir.dt.float32
    op_mult = mybir.AluOpType.mult
    op_add = mybir.AluOpType.add
    op_sub = mybir.AluOpType.subtract
    ACT = mybir.ActivationFunctionType

    nc = bacc.Bacc("TRN2", target_bir_lowering=False, debug=False,
                   num_devices=N_CORES)
    ed_d = nc.dram_tensor("ed", [P, N_CH * 12], dt, kind="ExternalInput")
    aux_d = nc.dram_tensor("aux", [P, N_CH + NQ + N_RBF], dt,
                           kind="ExternalInput")
    oh_d = nc.dram_tensor("oh8", [P, N_CH * NQ], mybir.dt.uint8,
                          kind="ExternalInput")
    out_d = nc.dram_tensor("out", [N_CH * NT, N_RBF * NF * NC9], dt,
                           kind="ExternalOutput")
    dbg = {}
    if debug:
        for nm, w in [("ang", N_CH * NM), ("radial", N_CH * N_RBF),
                      ("enc", N_CH * NC9), ("lhsT", N_CH * NQ),
                      ("A", N_CH * NM * NC9), ("ln", N_CH),
                      ("sinr", N_CH * N_RBF),
                      ("wfac", N_CH), ("fcv", N_CH)]:
            dbg[nm] = nc.dram_tensor("dbg_" + nm, [P, w], dt,
                                     kind="ExternalOutput")

    plan = build_plan()
    slot_of, n_slots = _alloc_slots(plan)

    with TileContext(nc) as tc:
        with (
            tc.tile_pool(name="io", bufs=1) as io,
            tc.tile_pool(name="apool", bufs=1) as apl,
            tc.tile_pool(name="psum", bufs=4, space="PSUM") as pp,
        ):
            ep_cm = tc.tile_pool(name="edge", bufs=1)
            ep = ep_cm.__enter__()
            ed = io.tile([P, N_CH * 12], dt)
            aux = io.tile([P, N_CH + NQ + N_RBF], dt)
            nc.sync.dma_start(out=ed[:, :], in_=ed_d[:, :])
            nc.sync.dma_start(out=aux[:, :], in_=aux_d[:, :])
            rloc = aux[:, 0:N_CH]
            cpat = aux[:, N_CH:N_CH + NQ]
            cn8 = aux[:, N_CH + NQ:N_CH + NQ + N_RBF]

            edv = ed[:, :].rearrange("p (ch t) -> p ch t", t=12)
            pos_s = edv[:, :, 0:3]
            pos_r = edv[:, :, 3:6]
            emb_s = edv[:, :, 6:9]
            emb_r = edv[:, :, 9:12]

            d = ep.tile([P, N_CH * 3], dt)
            dv = d[:, :].rearrange("p (ch t) -> p ch t", t=3)
            nc.vector.tensor_tensor(out=dv, in0=pos_r, in1=pos_s, op=op_sub)
            dsq = ep.tile([P, N_CH * 3], dt)
            dsqv = dsq[:, :].rearrange("p (ch t) -> p ch t", t=3)
            nc.vector.tensor_tensor(out=dsqv, in0=dv, in1=dv, op=op_mult)
            l2 = ep.tile([P, N_CH], dt)
            nc.vector.tensor_reduce(out=l2[:, :], in_=dsqv,
                                    axis=mybir.AxisListType.X, op=op_add)
            ln = ep.tile([P, N_CH], dt)
            nc.scalar.activation(out=ln[:, :], in_=l2[:, :], func=ACT.Sqrt)
            le = ep.tile([P, N_CH], dt)
            nc.vector.tensor_scalar_add(le[:, :], ln[:, :], EPS)
            rinv = ep.tile([P, N_CH], dt)
            nc.vector.reciprocal(out=rinv[:, :], in_=le[:, :])
            unit = ep.tile([P, N_CH * 3], dt)
            unitv = unit[:, :].rearrange("p (ch t) -> p ch t", t=3)
            nc.vector.tensor_tensor(
                out=unitv, in0=dv,
                in1=rinv[:, :].unsqueeze(2).to_broadcast([P, N_CH, 3]),
                op=op_mult)

            # angular monomials [p, ch, 20]
            ang = ep.tile([P, N_CH * NM], dt)
            av = ang[:, :].rearrange("p (ch m) -> p ch m", m=NM)
            nc.vector.memset(av[:, :, 0:1], 1.0)
            nc.vector.tensor_copy(av[:, :, 1:4], unitv)
            nc.vector.tensor_tensor(
                out=av[:, :, 4:7],
                in0=av[:, :, 1:2].to_broadcast([P, N_CH, 3]),
                in1=av[:, :, 1:4], op=op_mult)
            nc.vector.tensor_tensor(
                out=av[:, :, 7:9],
                in0=av[:, :, 2:3].to_broadcast([P, N_CH, 2]),
                in1=av[:, :, 2:4], op=op_mult)
            nc.vector.tensor_tensor(
                out=av[:, :, 9:10], in0=av[:, :, 3:4], in1=av[:, :, 3:4],
                op=op_mult)
            nc.vector.tensor_tensor(
                out=av[:, :, 10:16],
                in0=av[:, :, 1:2].to_broadcast([P, N_CH, 6]),
                in1=av[:, :, 4:10], op=op_mult)
            nc.vector.tensor_tensor(
                out=av[:, :, 16:19],
                in0=av[:, :, 2:3].to_broadcast([P, N_CH, 3]),
                in1=av[:, :, 7:10], op=op_mult)
            nc.vector.tensor_tensor(
                out=av[:, :, 19:20], in0=av[:, :, 3:4], in1=av[:, :, 9:10],
                op=op_mult)

            # encoded [p, ch, 3, 3] (c = s*3 + r)
            enc = ep.tile([P, N_CH * NC9], dt)
            ev = enc[:, :].rearrange("p (ch a b) -> p ch a b", a=3, b=3)
            nc.vector.tensor_tensor(
                out=ev,
                in0=emb_s.unsqueeze(3).to_broadcast([P, N_CH, 3, 3]),
                in1=emb_r.unsqueeze(2).to_broadcast([P, N_CH, 3, 3]),
                op=op_mult)

            # radial [p, ch, 8] via Chebyshev recurrence on clamped angle
            lc = ep.tile([P, N_CH], dt)
            nc.vector.tensor_scalar_min(lc[:, :], ln[:, :], CUTOFF)
            th = ep.tile([P, N_CH], dt)
            nc.vector.tensor_scalar_mul(th[:, :], lc[:, :], math.pi / CUTOFF)
            hh = ep.tile([P, N_CH], dt)
            nc.vector.tensor_scalar_mul(hh[:, :], lc[:, :],
                                        math.pi / (2.0 * CUTOFF))
            s2 = ep.tile([P, N_CH], dt)
            nc.scalar.activation(out=s2[:, :], in_=hh[:, :], func=ACT.Sin)
            s2q = ep.tile([P, N_CH], dt)
            nc.scalar.activation(out=s2q[:, :], in_=s2[:, :], func=ACT.Square)
            c2 = ep.tile([P, N_CH], dt)
            nc.vector.tensor_scalar(c2[:, :], s2q[:, :], -4.0, 2.0,
                                    op_mult, op_add)
            sinr = ep.tile([P, N_CH * N_RBF], dt)
            sv = sinr[:, :].rearrange("p (ch r) -> p ch r", r=N_RBF)
            nc.scalar.activation(out=sv[:, :, 0], in_=th[:, :], func=ACT.Sin)
            nc.vector.tensor_tensor(out=sv[:, :, 1], in0=c2[:, :],
                                    in1=sv[:, :, 0], op=op_mult)
            for n in range(2, N_RBF):
                tmp_n = ep.tile([P, N_CH], dt, tag=f"cheb{n % 2}")
                nc.vector.tensor_tensor(out=tmp_n[:, :], in0=c2[:, :],
                                        in1=sv[:, :, n - 1], op=op_mult)
                nc.vector.tensor_tensor(out=sv[:, :, n], in0=tmp_n[:, :],
                                        in1=sv[:, :, n - 2], op=op_sub)
            # fc polynomial
            uu = ep.tile([P, N_CH], dt)
            nc.vector.tensor_scalar_mul(uu[:, :], ln[:, :], 1.0 / CUTOFF)
            u2 = ep.tile([P, N_CH], dt)
            nc.vector.tensor_tensor(out=u2[:, :], in0=uu[:, :], in1=uu[:, :],
                                    op=op_mult)
            u3 = ep.tile([P, N_CH], dt)
            nc.vector.tensor_tensor(out=u3[:, :], in0=u2[:, :], in1=uu[:, :],
                                    op=op_mult)
            u6 = ep.tile([P, N_CH], dt)
            nc.vector.tensor_tensor(out=u6[:, :], in0=u3[:, :], in1=u3[:, :],
                                    op=op_mult)
            t1 = ep.tile([P, N_CH], dt)
            nc.vector.tensor_scalar(t1[:, :], uu[:, :], -21.0, 48.0,
                                    op_mult, op_add)
            t2 = ep.tile([P, N_CH], dt)
            nc.vector.tensor_tensor(out=t2[:, :], in0=t1[:, :], in1=uu[:, :],
                                    op=op_mult)
            nc.vector.tensor_scalar_add(t2[:, :], t2[:, :], -28.0)
            fcv = ep.tile([P, N_CH], dt)
            nc.vector.tensor_tensor(out=fcv[:, :], in0=u6[:, :], in1=t2[:, :],
                                    op=op_mult)
            nc.vector.tensor_scalar_add(fcv[:, :], fcv[:, :], 1.0)
            msk = ep.tile([P, N_CH], dt)
            nc.vector.tensor_scalar(msk[:, :], ln[:, :], CUTOFF, None,
                                    mybir.AluOpType.is_lt)
            nc.vector.tensor_tensor(out=fcv[:, :], in0=fcv[:, :], in1=msk[:, :],
                                    op=op_mult)
            wfac = ep.tile([P, N_CH], dt)
            nc.vector.tensor_tensor(out=wfac[:, :], in0=fcv[:, :],
                                    in1=rinv[:, :], op=op_mult)
            nc.vector.tensor_scalar_mul(wfac[:, :], wfac[:, :], SQ2C)
            radial = ep.tile([P, N_CH * N_RBF], dt)
            radv = radial[:, :].rearrange("p (ch r) -> p ch r", r=N_RBF)
            nc.vector.tensor_tensor(
                out=radv, in0=sinr[:, :].rearrange("p (ch r) -> p ch r", r=N_RBF),
                in1=wfac[:, :].unsqueeze(2).to_broadcast([P, N_CH, N_RBF]),
                op=op_mult)

            # rhs slab [p, ch, 20m, 9c] ; lhsT slab [p, ch, 14n, 8r]
            rhs = ep.tile([P, N_CH * NM * NC9], dt)
            rv = rhs[:, :].rearrange("p (ch m c) -> p ch m c", m=NM, c=NC9)
            nc.vector.tensor_tensor(
                out=rv,
                in0=av.unsqueeze(3).to_broadcast([P, N_CH, NM, NC9]),
                in1=ev.rearrange("p ch a b -> p ch (a b)").unsqueeze(2)
                    .to_broadcast([P, N_CH, NM, NC9]),
                op=op_mult)
            ohf = ep.tile([P, N_CH * NQ], dt)
            nc.gpsimd.dma_start(out=ohf[:, :], in_=oh_d[:, :])
            lhsT = ep.tile([P, N_CH * NQ], dt)
            lv = lhsT[:, :].rearrange("p (ch n r) -> p ch n r", n=NT, r=N_RBF)
            nc.vector.tensor_tensor(
                out=lv,
                in0=ohf[:, :].rearrange("p (ch n r) -> p ch n r", n=NT,
                                        r=N_RBF),
                in1=radv.unsqueeze(2).to_broadcast([P, N_CH, NT, N_RBF]),
                op=op_mult)

            # scatter matmuls -> A slab [112, ch*180]
            A = apl.tile([P, N_CH * NM * NC9], dt)
            lhv = lhsT[:, :].rearrange("p (ch q) -> p ch q", q=NQ)
            rhv = rhs[:, :].rearrange("p (ch f) -> p ch f", f=NM * NC9)
            Avw = A[:, :].rearrange("p (ch f) -> p ch f", f=NM * NC9)
            for ch2 in range(N_CH // 2):
                pt = pp.tile([NQ, 2 * NM * NC9], dt)
                for k in range(2):
                    ch = ch2 * 2 + k
                    nc.tensor.matmul(
                        out=pt[:, k * 180:(k + 1) * 180],
                        lhsT=lhv[:, ch, :], rhs=rhv[:, ch, :],
                        start=True, stop=True)
                nc.scalar.copy(
                    out=Avw[:NQ, ch2 * 2:ch2 * 2 + 2, :].rearrange(
                        "p ch f -> p (ch f)"),
                    in_=pt[:, :])

            if debug:
                nc.sync.dma_start(out=dbg["ang"][:, :], in_=ang[:, :])
                nc.sync.dma_start(out=dbg["radial"][:, :], in_=radial[:, :])
                nc.sync.dma_start(out=dbg["enc"][:, :], in_=enc[:, :])
                nc.sync.dma_start(out=dbg["lhsT"][:, :], in_=lhsT[:, :])
                nc.sync.dma_start(out=dbg["A"][:, :], in_=A[:, :])
                nc.sync.dma_start(out=dbg["ln"][:, :], in_=ln[:, :])
                nc.sync.dma_start(out=dbg["sinr"][:, :], in_=sinr[:, :])
                nc.sync.dma_start(out=dbg["wfac"][:, :], in_=wfac[:, :])
                nc.sync.dma_start(out=dbg["fcv"][:, :], in_=fcv[:, :])
            # ---- symmetrization ----
            ep_cm.__exit__(None, None, None)
            sy_cm = tc.tile_pool(name="sym", bufs=1)
            sy = sy_cm.__enter__()
            feats = sy.tile([P, N_CH * NF * NC9], dt)
            Qs = sy.tile([P, N_CH * NM * NC9], dt)
            nc.scalar.activation(out=Qs[:NQ, :], in_=A[:NQ, :],
                                 func=ACT.Square)
            slots = sy.tile([P, n_slots * N_CH * NC9], dt)
            slotv = slots[:, :].rearrange("p (s ch c) -> p s ch c", s=n_slots,
                                          c=NC9)

            def plane(pid):
                if pid[0] == 'A':
                    m = pid[1]
                    return A[:NQ, :].rearrange(
                        "p (ch m c) -> p ch m c", m=NM, c=NC9)[:, :, pid[1], :]
                if pid[0] == 'Q':
                    return Qs[:NQ, :].rearrange(
                        "p (ch m c) -> p ch m c", m=NM, c=NC9)[:, :, pid[1], :]
                if pid[0] == 'F':
                    return feats[:NQ, :].rearrange(
                        "p (ch f c) -> p ch f c", f=NF, c=NC9)[:, :, pid[1], :]
                return slotv[:NQ, slot_of[pid], :, :]

            eng_tt = {'v': nc.vector, 'g': nc.vector}
            for op in plan.ops:
                eng, kind, dst = op[0], op[1], op[2]
                do = plane(dst)
                if kind in ('mul', 'add'):
                    nc_e = eng_tt.get(eng, nc.vector)
                    nc_e.tensor_tensor(out=do, in0=plane(op[3]),
                                       in1=plane(op[4]),
                                       op=op_mult if kind == 'mul' else op_add)
                elif kind == 'sq':
                    nc.scalar.activation(out=do, in_=plane(op[3]),
                                         func=ACT.Square)
                elif kind == 'wmul':
                    nc.scalar.activation(out=do, in_=plane(op[3]),
                                         func=ACT.Copy, scale=float(op[4]))
                elif kind == 'copy':
                    nc.scalar.copy(out=do, in_=plane(op[3]))

            # output DMA: feats [112=(14n,8r), ch*(11f*9c)] -> [ch*14, 792]
            src = feats[:NQ, :].rearrange("p (ch x) -> p ch x", x=NF * NC9)
            dst = out_d[:, :].rearrange("(ch n) (r x) -> n r ch x",
                                        ch=N_CH, r=N_RBF)
            nc.sync.dma_start(out=dst, in_=src)
            sy_cm.__exit__(None, None, None)
    nc.compile()
    return nc, plan


# ---------------- host side -------------------------------------------------
def _host_prep(inputs):
    pos = np.ascontiguousarray(inputs['positions'], np.float32)
    W = np.asarray(inputs['W_embed'], np.float32)
    an = np.asarray(inputs['atomic_numbers'])
    ei = np.asarray(inputs['edge_index'])
    zs = np.asarray(ZS, an.dtype)
    onehot = (an[:, None] == zs[None, :]).astype(np.float32)
    emb = onehot @ W
    send, recv = ei[0], ei[1]
    order = np.argsort(recv, kind='stable')
    send, recv = send[order], recv[order]
    counts = np.bincount(recv, minlength=N_NODES)
    starts = np.concatenate([[0], np.cumsum(counts)])
    in_maps = []
    chunk_meta = []
    cpat = np.repeat(np.arange(NT, dtype=np.float32), N_RBF)[None, :].repeat(P, 0)
    cn8 = (np.arange(1, N_RBF + 1, dtype=np.float32) * np.pi / CUTOFF
           )[None, :].repeat(P, 0)
    for core in range(N_CORES):
        n0, n1 = core * PER, (core + 1) * PER
        chunks = []
        node = n0
        while node < n1:
            base = node
            e_lo = starts[node]
            while (node < n1 and node - base < NT
                   and starts[node + 1] - e_lo <= P):
                node += 1
            assert node > base, f"node {base} degree > {P}"
            chunks.append((int(e_lo), int(starts[node]), int(base)))
        assert len(chunks) <= N_CH, f"core {core}: {len(chunks)} chunks > {N_CH}"
        ed = np.zeros((P, N_CH, 12), np.float32)
        rloc = np.zeros((P, N_CH), np.float32)
        for ci, (lo, hi, base) in enumerate(chunks):
            k = hi - lo
            es, er = send[lo:hi], recv[lo:hi]
            ed[:k, ci, 0:3] = pos[es]
            ed[:k, ci, 3:6] = pos[er]
            ed[:k, ci, 6:9] = emb[es]
            ed[:k, ci, 9:12] = emb[er]
            rloc[:k, ci] = (er - base).astype(np.float32)
        aux = np.concatenate([rloc, cpat, cn8], axis=1)
        oh8 = (rloc[:, :, None] ==
               np.floor(np.arange(NQ, dtype=np.float32) / N_RBF)[None, None, :]
               ).astype(np.uint8)
        in_maps.append({
            "ed": np.ascontiguousarray(ed.reshape(P, N_CH * 12)),
            "aux": np.ascontiguousarray(aux),
            "oh8": np.ascontiguousarray(oh8.reshape(P, N_CH * NQ)),
        })
        chunk_meta.append(chunks)
    return in_maps, chunk_meta


_NC_CACHE = [None]
_IN_MAPS_CACHE = [None]


def kernel(**inputs):
    from concourse.bass_utils import run_bass_kernel_spmd
    nc, _plan = _build_nc()
    in_maps, chunk_meta = _host_prep(inputs)
    _NC_CACHE[0] = nc
    _IN_MAPS_CACHE[0] = in_maps
    res = run_bass_kernel_spmd(nc, in_maps, core_ids=list(range(N_CORES)))
    out = np.zeros((N_NODES, N_RBF, NF, NC9), np.float32)
    for core in range(N_CORES):
        slab = res.results[core]["out"].reshape(N_CH, NT, N_RBF, NF, NC9)
        n0, n1 = core * PER, (core + 1) * PER
        chunks = chunk_meta[core]
        for ci, (lo, hi, base) in enumerate(chunks):
            nxt = chunks[ci + 1][2] if ci + 1 < len(chunks) else n1
            out[base:nxt] = slab[ci, :nxt - base]
    return out



# revision 4
# speedup vs baseline: 2.5604x; 2.5604x over previous
"""CACE GNN message-passing kernel for 8 trn2 NeuronCores (v2).

Node-parallel sharding (625 nodes/core), edges sorted by receiver and packed
into <=128-edge x <=16-node chunks. Key structure exploited: within a segment
all edges share the receiver node, so A[n,r,m,(cs,cr)] factorizes as
Atilde[n,r,m,cs] * emb_recv[n,cr]; the nu=2..4 symmetrization runs on 3
channels instead of 9 and the result is expanded by emb_recv^nu at the end.

Per core:
  1. edge factors: radial[8] (Bessel*cutoff via one wide ACT-Sin), unit vec,
     rhs = ordered-monomial basis (1,V,S9,T27) x emb_send  [128e, 120] f32,
     lhsT = onehot(node,16) x radial [128e, 128] f32 (on Pool engine),
  2. one f32 matmul per chunk -> PSUM, ACT copies -> Atilde slab (bf16),
  3. dense bf16 symmetrization (ordered index sets, big strided tensor ops,
     tree adds) -> 11 features x 3 cs channels,
  4. f32 expansion by emb_recv^nu(f) -> [128p, ch, 11, 9] -> DMA out.
"""
import math
import functools
import numpy as np

# ---------------- problem constants ----------------------------------------
N_NODES, N_EDGES = 5000, 50000
N_RBF = 8
CUTOFF = 5.5
EPS = 1e-9
ZS = [1, 6, 7, 8]
N_CORES = 8
PER = N_NODES // N_CORES          # 625 nodes per core
NT = 16                           # nodes per chunk
N_CH = 53                         # chunks per core (padded; max measured 53)
P = 128                           # edges per chunk (partitions)
NQ = NT * N_RBF                   # 128 = lhsT free dim
MB = 40                           # ordered monomial planes: 1 + 3 + 9 + 27
NF = 11
SQ2C = math.sqrt(2.0 / CUTOFF)


# ---------------- device kernel build --------------------------------------
@functools.lru_cache(maxsize=2)
def _build_nc():
    import concourse.bass as bass
    import concourse.bacc as bacc
    import concourse.mybir as mybir
    from concourse.tile import TileContext

    f32 = mybir.dt.float32
    bf16 = mybir.dt.bfloat16
    MUL = mybir.AluOpType.mult
    ADD = mybir.AluOpType.add
    SUB = mybir.AluOpType.subtract
    ACT = mybir.ActivationFunctionType
    AX = mybir.AxisListType.X

    nc = bacc.Bacc("TRN2", target_bir_lowering=False, debug=False,
                   num_devices=N_CORES)
    ed_d = nc.dram_tensor("ed", [P, N_CH * 6], f32, kind="ExternalInput")
    oh_d = nc.dram_tensor("oh", [P, N_CH * NT], f32, kind="ExternalInput")
    ebr_d = nc.dram_tensor("ebr", [P, N_CH * 3], f32, kind="ExternalInput")
    out_d = nc.dram_tensor("out", [P, N_CH * NF * 9], f32,
                           kind="ExternalOutput")

    with TileContext(nc) as tc:
        with (
            tc.tile_pool(name="io", bufs=1) as io,
            tc.tile_pool(name="work", bufs=1) as wk,
            tc.tile_pool(name="psum", bufs=3, space="PSUM") as pp,
        ):
            ed = io.tile([P, N_CH * 6], f32)
            oh = io.tile([P, N_CH * NT], f32)
            ebr = io.tile([P, N_CH * 3], f32)
            nc.sync.dma_start(out=ed[:, :], in_=ed_d[:, :])
            nc.sync.dma_start(out=oh[:, :], in_=oh_d[:, :])
            nc.sync.dma_start(out=ebr[:, :], in_=ebr_d[:, :])

            edv = ed[:, :].rearrange("p (ch t) -> p ch t", t=6)
            vec = edv[:, :, 0:3]
            embS = edv[:, :, 3:6]

            # ---- geometry (f32) ----
            dsq = wk.tile([P, N_CH * 3], f32)
            dsqv = dsq[:, :].rearrange("p (ch t) -> p ch t", t=3)
            nc.vector.tensor_tensor(out=dsqv, in0=vec, in1=vec, op=MUL)
            l2 = wk.tile([P, N_CH], f32)
            nc.vector.tensor_reduce(out=l2[:, :], in_=dsqv, axis=AX, op=ADD)
            ln = wk.tile([P, N_CH], f32)
            nc.scalar.activation(out=ln[:, :], in_=l2[:, :], func=ACT.Sqrt)
            le = wk.tile([P, N_CH], f32)
            nc.vector.tensor_scalar_add(le[:, :], ln[:, :], EPS)
            rinv = wk.tile([P, N_CH], f32)
            nc.vector.reciprocal(out=rinv[:, :], in_=le[:, :])
            unit = wk.tile([P, N_CH * 3], f32)
            unitv = unit[:, :].rearrange("p (ch t) -> p ch t", t=3)
            nc.vector.tensor_tensor(
                out=unitv, in0=vec,
                in1=rinv[:, :].unsqueeze(2).to_broadcast([P, N_CH, 3]),
                op=MUL)

            # ---- radial (f32): sin(n*pi*l/C)*sqrt(2/C)*fc(l)/(l+eps) ----
            lc = wk.tile([P, N_CH], f32)
            nc.vector.tensor_scalar_min(lc[:, :], ln[:, :], CUTOFF)
            nth = wk.tile([P, N_CH * N_RBF], f32)
            nthv = nth[:, :].rearrange("p (ch r) -> p ch r", r=N_RBF)
            # n*pi/C per rbf via iota-style constant: build with tensor_scalar
            # on broadcast lc then per-column scale is awkward; instead use
            # 8 scalar mults fused as one op with a constant vector from host?
            # simplest: one tensor_scalar per rbf would be 8 ops; use
            # broadcast multiply against a constant tile built by memsets.
            cvec = wk.tile([P, N_RBF], f32)
            for r in range(N_RBF):
                nc.vector.memset(cvec[:, r:r + 1], (r + 1) * math.pi / CUTOFF)
            nc.vector.tensor_tensor(
                out=nthv,
                in0=lc[:, :].unsqueeze(2).to_broadcast([P, N_CH, N_RBF]),
                in1=cvec[:, :].unsqueeze(1).to_broadcast([P, N_CH, N_RBF]),
                op=MUL)
            sinr = wk.tile([P, N_CH * N_RBF], f32)
            nc.scalar.activation(out=sinr[:, :], in_=nth[:, :], func=ACT.Sin)
            # fc(u) = 1 - 28u^6 + 48u^7 - 21u^8, u = ln/C (clamped by mask)
            uu = wk.tile([P, N_CH], f32)
            nc.vector.tensor_scalar_mul(uu[:, :], ln[:, :], 1.0 / CUTOFF)
            u2 = wk.tile([P, N_CH], f32)
            nc.vector.tensor_tensor(out=u2[:, :], in0=uu[:, :], in1=uu[:, :],
                                    op=MUL)
            u3 = wk.tile([P, N_CH], f32)
            nc.vector.tensor_tensor(out=u3[:, :], in0=u2[:, :], in1=uu[:, :],
                                    op=MUL)
            u6 = wk.tile([P, N_CH], f32)
            nc.vector.tensor_tensor(out=u6[:, :], in0=u3[:, :], in1=u3[:, :],
                                    op=MUL)
            t1 = wk.tile([P, N_CH], f32)
            nc.vector.tensor_scalar(t1[:, :], uu[:, :], -21.0, 48.0, MUL, ADD)
            t2 = wk.tile([P, N_CH], f32)
            nc.vector.tensor_tensor(out=t2[:, :], in0=t1[:, :], in1=uu[:, :],
                                    op=MUL)
            nc.vector.tensor_scalar_add(t2[:, :], t2[:, :], -28.0)
            fcv = wk.tile([P, N_CH], f32)
            nc.vector.tensor_tensor(out=fcv[:, :], in0=u6[:, :], in1=t2[:, :],
                                    op=MUL)
            nc.vector.tensor_scalar_add(fcv[:, :], fcv[:, :], 1.0)
            msk = wk.tile([P, N_CH], f32)
            nc.vector.tensor_scalar(msk[:, :], ln[:, :], CUTOFF, None,
                                    mybir.AluOpType.is_lt)
            nc.vector.tensor_tensor(out=fcv[:, :], in0=fcv[:, :],
                                    in1=msk[:, :], op=MUL)
            wfac = wk.tile([P, N_CH], f32)
            nc.vector.tensor_tensor(out=wfac[:, :], in0=fcv[:, :],
                                    in1=rinv[:, :], op=MUL)
            nc.vector.tensor_scalar_mul(wfac[:, :], wfac[:, :], SQ2C)
            radial = wk.tile([P, N_CH * N_RBF], f32)
            radv = radial[:, :].rearrange("p (ch r) -> p ch r", r=N_RBF)
            nc.vector.tensor_tensor(
                out=radv,
                in0=sinr[:, :].rearrange("p (ch r) -> p ch r", r=N_RBF),
                in1=wfac[:, :].unsqueeze(2).to_broadcast([P, N_CH, N_RBF]),
                op=MUL)

            # ---- lhsT = onehot x radial (f32) on Pool, split in two ----
            lhsT = wk.tile([P, N_CH * NQ], f32)
            lv = lhsT[:, :].rearrange("p (ch n r) -> p ch n r", n=NT, r=N_RBF)
            ohv = oh[:, :].rearrange("p (ch n) -> p ch n", n=NT)
            H1 = 27
            for lo, hi in ((0, H1), (H1, N_CH)):
                nc.gpsimd.tensor_tensor(
                    out=lv[:, lo:hi],
                    in0=ohv[:, lo:hi].unsqueeze(3).to_broadcast(
                        [P, hi - lo, NT, N_RBF]),
                    in1=radv[:, lo:hi].unsqueeze(2).to_broadcast(
                        [P, hi - lo, NT, N_RBF]),
                    op=MUL)

            # ---- rhs: ordered basis x embS, cascaded (f32) ----
            # rhs layout per chunk: [MB=40, 3cs]: plane0=embS, 1..3=V*embS,
            # 4..12=S9*embS, 13..39=T27*embS
            S9 = wk.tile([P, N_CH * 9], f32)
            s9v = S9[:, :].rearrange("p (ch a b) -> p ch a b", a=3, b=3)
            nc.vector.tensor_tensor(
                out=s9v,
                in0=unitv.unsqueeze(3).to_broadcast([P, N_CH, 3, 3]),
                in1=unitv.unsqueeze(2).to_broadcast([P, N_CH, 3, 3]),
                op=MUL)
            rhs = wk.tile([P, N_CH * MB * 3], f32)
            rv = rhs[:, :].rearrange("p (ch m c) -> p ch m c", m=MB, c=3)
            nc.scalar.copy(out=rv[:, :, 0, :], in_=embS)
            # VE = V x embS -> rhs planes 1..3
            nc.vector.tensor_tensor(
                out=rv[:, :, 1:4, :],
                in0=unitv.unsqueeze(3).to_broadcast([P, N_CH, 3, 3]),
                in1=embS.unsqueeze(2).to_broadcast([P, N_CH, 3, 3]),
                op=MUL)
            # S9 x embS -> planes 4..12  (= V x VE)
            nc.vector.tensor_tensor(
                out=rv[:, :, 4:13, :].rearrange("p ch (a b) c -> p ch a b c",
                                                a=3),
                in0=unitv.unsqueeze(3).unsqueeze(4).to_broadcast(
                    [P, N_CH, 3, 3, 3]),
                in1=rv[:, :, 1:4, :].rearrange("p ch b c -> p ch b c")
                    .unsqueeze(2).to_broadcast([P, N_CH, 3, 3, 3]),
                op=MUL)
            # T27 x embS -> planes 13..39 (= S9 x VE)
            nc.vector.tensor_tensor(
                out=rv[:, :, 13:40, :].rearrange(
                    "p ch (ab cc) c -> p ch ab cc c", ab=9),
                in0=s9v.rearrange("p ch a b -> p ch (a b)")
                    .unsqueeze(3).unsqueeze(4).to_broadcast(
                        [P, N_CH, 9, 3, 3]),
                in1=rv[:, :, 1:4, :].unsqueeze(2).to_broadcast(
                    [P, N_CH, 9, 3, 3]),
                op=MUL)

            # ---- matmuls (f32) + ACT psum->sbuf copies (bf16 A slab) ----
            A = wk.tile([P, N_CH * MB * 3], bf16)
            Av = A[:, :].rearrange("p (ch m c) -> p ch m c", m=MB, c=3)
            rflat = rhs[:, :].rearrange("p (ch f) -> p ch f", f=MB * 3)
            lflat = lhsT[:, :].rearrange("p (ch q) -> p ch q", q=NQ)
            Aflat = A[:, :].rearrange("p (ch f) -> p ch f", f=MB * 3)
            GW = 4  # chunks per psum tile (480 f32 cols = 1 bank x ... )
            n_grp = (N_CH + GW - 1) // GW
            for g in range(n_grp):
                c0 = g * GW
                c1 = min(N_CH, c0 + GW)
                pt = pp.tile([P, GW * MB * 3], f32)
                for ch in range(c0, c1):
                    k = ch - c0
                    nc.tensor.matmul(
                        out=pt[:, k * 120:(k + 1) * 120],
                        lhsT=lflat[:, ch, :], rhs=rflat[:, ch, :],
                        start=True, stop=True)
                nc.scalar.copy(
                    out=Aflat[:, c0:c1, :].rearrange("p ch f -> p (ch f)"),
                    in_=pt[:, :(c1 - c0) * 120])

            # ---- symmetrization (bf16, dense ordered ops) ----
            CH = N_CH
            with nc.allow_low_precision(reason="bf16 feature pipeline"):
                # Q = squares of planes 1..39
                Q = wk.tile([P, CH * 39 * 3], bf16)
                Qv = Q[:, :].rearrange("p (ch m c) -> p ch m c", m=39, c=3)
                a139 = Av[:, :, 1:40, :]
                nc.vector.tensor_tensor(out=Qv, in0=a139, in1=a139, op=MUL)

                Ft = wk.tile([P, CH * NF * 3], bf16)
                Fv = Ft[:, :].rearrange("p (ch f c) -> p ch f c", f=NF, c=3)

                V_ = Av[:, :, 1:4, :]                      # [p,ch,3,3c]
                S_ = Av[:, :, 4:13, :]                     # [p,ch,9,3c]
                Sm = S_.rearrange("p ch (a b) c -> p ch a b c", a=3)
                T_ = Av[:, :, 13:40, :]                    # [p,ch,27,3c]
                Tm = T_.rearrange("p ch (a b cc) c -> p ch a b cc c", a=3,
                                  b=3)
                Tab = T_.rearrange("p ch (ab cc) c -> p ch ab cc c", ab=9)

                def c3(dst, src):
                    """dst = src[...,0,:]+src[...,1,:]+src[...,2,:] where the
                    contraction axis is the 3rd-from-last of src view."""
                    nc.vector.tensor_tensor(out=dst, in0=src[0], in1=src[1],
                                            op=ADD)
                    nc.vector.tensor_tensor(out=dst, in0=dst, in1=src[2],
                                            op=ADD)

                # nu1 (F0): copy plane 0
                nc.scalar.copy(out=Fv[:, :, 0, :], in_=Av[:, :, 0, :])

                # nu2_1 = sum V^2 -> F1
                q1 = Qv[:, :, 0:3, :]
                c3(Fv[:, :, 1, :], [q1[:, :, i, :] for i in range(3)])
                # nu2_2 = sum S9^2 -> F2
                q2 = Qv[:, :, 3:12, :].rearrange("p ch (a b) c -> p ch a b c",
                                                 a=3)
                t9 = wk.tile([P, CH * 3 * 3], bf16)
                t9v = t9[:, :].rearrange("p (ch b c) -> p ch b c", b=3, c=3)
                c3(t9v, [q2[:, :, i, :, :] for i in range(3)])
                c3(Fv[:, :, 2, :], [t9v[:, :, i, :] for i in range(3)])
                # nu2_3 = sum T27^2 -> F3
                q3 = Qv[:, :, 12:39, :].rearrange(
                    "p ch (a bc) c -> p ch a bc c", a=3)
                t27a = wk.tile([P, CH * 9 * 3], bf16)
                t27av = t27a[:, :].rearrange("p (ch bc c) -> p ch bc c", bc=9,
                                             c=3)
                c3(t27av, [q3[:, :, i, :, :] for i in range(3)])
                t27b = t27av.rearrange("p ch (b cc) c -> p ch b cc c", b=3)
                c3(t9v, [t27b[:, :, i, :, :] for i in range(3)])
                c3(Fv[:, :, 3, :], [t9v[:, :, i, :] for i in range(3)])

                # u_a = sum_b S_ab V_b   [p,ch,3,3c]
                u9 = wk.tile([P, CH * 9 * 3], bf16)
                u9v = u9[:, :].rearrange("p (ch a b c) -> p ch a b c", a=3,
                                         b=3, c=3)
                nc.vector.tensor_tensor(
                    out=u9v, in0=Sm,
                    in1=V_.unsqueeze(2).to_broadcast([P, CH, 3, 3, 3]),
                    op=MUL)
                u_ = wk.tile([P, CH * 3 * 3], bf16)
                uv = u_[:, :].rearrange("p (ch a c) -> p ch a c", a=3, c=3)
                c3(uv, [u9v[:, :, :, i, :] for i in range(3)])

                # z_c = sum_ab T_abc S_ab  [p,ch,3,3c]
                tz = wk.tile([P, CH * 27 * 3], bf16)
                tzv = tz[:, :].rearrange("p (ch ab cc c) -> p ch ab cc c",
                                         ab=9, cc=3, c=3)
                nc.vector.tensor_tensor(
                    out=tzv, in0=Tab,
                    in1=S_.unsqueeze(3).to_broadcast([P, CH, 9, 3, 3]),
                    op=MUL)
                tz2 = tzv.rearrange("p ch (a b) cc c -> p ch a b cc c", a=3)
                z9 = wk.tile([P, CH * 9 * 3], bf16)
                z9v = z9[:, :].rearrange("p (ch b cc c) -> p ch b cc c", b=3,
                                         cc=3, c=3)
                c3(z9v, [tz2[:, :, i, :, :, :] for i in range(3)])
                z_ = wk.tile([P, CH * 3 * 3], bf16)
                zv = z_[:, :].rearrange("p (ch cc c) -> p ch cc c", cc=3, c=3)
                c3(zv, [z9v[:, :, i, :, :] for i in range(3)])

                # P2_bc = sum_a V_a T_abc  [p,ch,9,3c]
                pv = wk.tile([P, CH * 27 * 3], bf16)
                pvv = pv[:, :].rearrange("p (ch a bc c) -> p ch a bc c", a=3,
                                         bc=9, c=3)
                nc.vector.tensor_tensor(
                    out=pvv,
                    in0=Tm.rearrange("p ch a b cc c -> p ch a (b cc) c"),
                    in1=V_.unsqueeze(3).to_broadcast([P, CH, 3, 9, 3]),
                    op=MUL)
                p2 = wk.tile([P, CH * 9 * 3], bf16)
                p2v = p2[:, :].rearrange("p (ch bc c) -> p ch bc c", bc=9,
                                         c=3)
                c3(p2v, [pvv[:, :, i, :, :] for i in range(3)])

                # S2_ab = sum_k S_ak S_kb  [p,ch,9,3c]
                ss = wk.tile([P, CH * 27 * 3], bf16)
                ssv = ss[:, :].rearrange("p (ch a k b c) -> p ch a k b c",
                                         a=3, k=3, b=3, c=3)
                nc.vector.tensor_tensor(
                    out=ssv,
                    in0=Sm.unsqueeze(4).to_broadcast([P, CH, 3, 3, 3, 3]),
                    in1=Sm.unsqueeze(2).to_broadcast([P, CH, 3, 3, 3, 3]),
                    op=MUL)
                s2 = wk.tile([P, CH * 9 * 3], bf16)
                s2v = s2[:, :].rearrange("p (ch a b c) -> p ch a b c", a=3,
                                         b=3, c=3)
                c3(s2v, [ssv[:, :, :, i, :, :] for i in range(3)])

                # trS3 = sum_ab S2_ab S_ab -> F4
                w9 = wk.tile([P, CH * 9 * 3], bf16)
                w9v = w9[:, :].rearrange("p (ch ab c) -> p ch ab c", ab=9,
                                         c=3)
                nc.vector.tensor_tensor(
                    out=w9v,
                    in0=s2v.rearrange("p ch a b c -> p ch (a b) c"),
                    in1=S_, op=MUL)
                w9m = w9v.rearrange("p ch (a b) c -> p ch a b c", a=3)
                c3(t9v, [w9m[:, :, i, :, :] for i in range(3)])
                c3(Fv[:, :, 4, :], [t9v[:, :, i, :] for i in range(3)])

                # M_cd (c<=d runs) = sum_ab T_abc T_abd
                mm = wk.tile([P, CH * 6 * 3], bf16)
                mmv = mm[:, :].rearrange("p (ch k c) -> p ch k c", k=6, c=3)
                moff = [0, 3, 5]
                prod = wk.tile([P, CH * 54 * 3], bf16)
                for c in range(3):
                    nd = 3 - c
                    pr = prod[:, :].rearrange("p (ch x) -> p ch x", x=54 * 3)[
                        :, :, :9 * nd * 3].rearrange(
                        "p ch (ab d c2) -> p ch ab d c2", ab=9, d=nd)
                    nc.vector.tensor_tensor(
                        out=pr,
                        in0=Tab[:, :, :, c, :].unsqueeze(3).to_broadcast(
                            [P, CH, 9, nd, 3]),
                        in1=Tab[:, :, :, c:3, :], op=MUL)
                    pr2 = pr.rearrange("p ch (a b) d c2 -> p ch a b d c2",
                                       a=3)
                    tmp = wk.tile([P, CH * 9 * 3], bf16, tag="mtmp")
                    tmpv = tmp[:, :].rearrange("p (ch b d c2) -> p ch b d c2",
                                               b=3, d=3, c2=3)[:, :, :, :nd, :]
                    c3(tmpv, [pr2[:, :, i, :, :, :] for i in range(3)])
                    c3(mmv[:, :, moff[c]:moff[c] + nd, :],
                       [tmpv[:, :, i, :, :] for i in range(3)])

                # nu3_2 = 2*sum_{c<=d} M_cd S_cd - sum_c M_cc S_cc -> F5
                # S planes for (c,d) c<=d: S9 idx {0,1,2},{4,5},{8}
                q6 = wk.tile([P, CH * 6 * 3], bf16)
                q6v = q6[:, :].rearrange("p (ch k c) -> p ch k c", k=6, c=3)
                nc.vector.tensor_tensor(out=q6v[:, :, 0:3, :],
                                        in0=mmv[:, :, 0:3, :],
                                        in1=S_[:, :, 0:3, :], op=MUL)
                nc.vector.tensor_tensor(out=q6v[:, :, 3:5, :],
                                        in0=mmv[:, :, 3:5, :],
                                        in1=S_[:, :, 4:6, :], op=MUL)
                nc.vector.tensor_tensor(out=q6v[:, :, 5:6, :],
                                        in0=mmv[:, :, 5:6, :],
                                        in1=S_[:, :, 8:9, :], op=MUL)
                sall = wk.tile([P, CH * 3], bf16)
                sallv = sall[:, :].rearrange("p (ch c) -> p ch c", c=3)
                t3a = wk.tile([P, CH * 2 * 3], bf16)
                t3av = t3a[:, :].rearrange("p (ch k c) -> p ch k c", k=2, c=3)
                nc.vector.tensor_tensor(out=t3av, in0=q6v[:, :, 0:2, :],
                                        in1=q6v[:, :, 2:4, :], op=ADD)
                nc.vector.tensor_tensor(out=t3av[:, :, 0, :],
                                        in0=t3av[:, :, 0, :],
                                        in1=q6v[:, :, 4, :], op=ADD)
                nc.vector.tensor_tensor(out=t3av[:, :, 1, :],
                                        in0=t3av[:, :, 1, :],
                                        in1=q6v[:, :, 5, :], op=ADD)
                nc.vector.tensor_tensor(out=sallv, in0=t3av[:, :, 0, :],
                                        in1=t3av[:, :, 1, :], op=ADD)
                sdia = wk.tile([P, CH * 3], bf16)
                sdiav = sdia[:, :].rearrange("p (ch c) -> p ch c", c=3)
                nc.vector.tensor_tensor(out=sdiav, in0=q6v[:, :, 0, :],
                                        in1=q6v[:, :, 3, :], op=ADD)
                nc.vector.tensor_tensor(out=sdiav, in0=sdiav,
                                        in1=q6v[:, :, 5, :], op=ADD)
                nc.vector.scalar_tensor_tensor(
                    out=Fv[:, :, 5, :], in0=sallv, scalar=2.0, in1=sdiav,
                    op0=MUL, op1=SUB)

                # nu4_1 = sum u^2 -> F6 ; nu4_2 = sum u z -> F7,F9 ;
                # nu4_5 = sum z^2 -> F10 ; nu4_3 = sum P2^2 -> F8
                uu3 = wk.tile([P, CH * 3 * 3], bf16)
                uu3v = uu3[:, :].rearrange("p (ch a c) -> p ch a c", a=3, c=3)
                nc.vector.tensor_tensor(out=uu3v, in0=uv, in1=uv, op=MUL)
                c3(Fv[:, :, 6, :], [uu3v[:, :, i, :] for i in range(3)])
                nc.vector.tensor_tensor(out=uu3v, in0=uv, in1=zv, op=MUL)
                c3(Fv[:, :, 7, :], [uu3v[:, :, i, :] for i in range(3)])
                nc.scalar.copy(out=Fv[:, :, 9, :], in_=Fv[:, :, 7, :])
                nc.vector.tensor_tensor(out=uu3v, in0=zv, in1=zv, op=MUL)
                c3(Fv[:, :, 10, :], [uu3v[:, :, i, :] for i in range(3)])
                pp9 = wk.tile([P, CH * 9 * 3], bf16)
                pp9v = pp9[:, :].rearrange("p (ch bc c) -> p ch bc c", bc=9,
                                           c=3)
                nc.vector.tensor_tensor(out=pp9v, in0=p2v, in1=p2v, op=MUL)
                pp3 = pp9v.rearrange("p ch (b cc) c -> p ch b cc c", b=3)
                c3(t9v, [pp3[:, :, i, :, :] for i in range(3)])
                c3(Fv[:, :, 8, :], [t9v[:, :, i, :] for i in range(3)])

            # ---- expansion by emb_recv^nu(f) (f32) ----
            ebv = ebr[:, :].rearrange("p (ch c) -> p ch c", c=3)
            e2 = wk.tile([P, N_CH * 3], f32)
            e2v = e2[:, :].rearrange("p (ch c) -> p ch c", c=3)
            nc.vector.tensor_tensor(out=e2v, in0=ebv, in1=ebv, op=MUL)
            e3 = wk.tile([P, N_CH * 3], f32)
            e3v = e3[:, :].rearrange("p (ch c) -> p ch c", c=3)
            nc.vector.tensor_tensor(out=e3v, in0=e2v, in1=ebv, op=MUL)
            e4 = wk.tile([P, N_CH * 3], f32)
            e4v = e4[:, :].rearrange("p (ch c) -> p ch c", c=3)
            nc.vector.tensor_tensor(out=e4v, in0=e2v, in1=e2v, op=MUL)

            outt = wk.tile([P, N_CH * NF * 9], f32)
            ov = outt[:, :].rearrange("p (ch f cs cr) -> p ch f cs cr", f=NF,
                                      cs=3, cr=3)
            runs = [(0, 1, ebv), (1, 4, e2v), (4, 6, e3v), (6, 11, e4v)]
            for f0, f1, ep in runs:
                nf_ = f1 - f0
                nc.vector.tensor_tensor(
                    out=ov[:, :, f0:f1, :, :],
                    in0=Fv[:, :, f0:f1, :].unsqueeze(4).to_broadcast(
                        [P, N_CH, nf_, 3, 3]),
                    in1=ep.unsqueeze(2).unsqueeze(3).to_broadcast(
                        [P, N_CH, nf_, 3, 3]),
                    op=MUL)
                nc.sync.dma_start(
                    out=out_d[:, :].rearrange(
                        "p (ch f x) -> p ch f x", f=NF, x=9)[:, :, f0:f1, :],
                    in_=ov[:, :, f0:f1, :, :].rearrange(
                        "p ch f cs cr -> p ch f (cs cr)"))
    nc.compile()
    return nc, None


# ---------------- host side -------------------------------------------------
def _host_prep(inputs):
    pos = np.ascontiguousarray(inputs['positions'], np.float32)
    W = np.asarray(inputs['W_embed'], np.float32)
    an = np.asarray(inputs['atomic_numbers'])
    ei = np.asarray(inputs['edge_index'])
    shifts = np.asarray(inputs.get('shifts'), np.float32)
    zs = np.asarray(ZS, an.dtype)
    onehot = (an[:, None] == zs[None, :]).astype(np.float32)
    emb = onehot @ W
    send, recv = ei[0], ei[1]
    order = np.argsort(recv, kind='stable')
    send_s, recv_s = send[order], recv[order]
    vec_all = pos[recv_s] - pos[send_s] + shifts[order]
    embS_all = emb[send_s]
    counts = np.bincount(recv_s, minlength=N_NODES)
    starts = np.concatenate([[0], np.cumsum(counts)])
    in_maps = []
    chunk_meta = []
    for core in range(N_CORES):
        n0, n1 = core * PER, (core + 1) * PER
        chunks = []
        node = n0
        while node < n1:
            base = node
            e_lo = starts[node]
            while (node < n1 and node - base < NT
                   and starts[node + 1] - e_lo <= P):
                node += 1
            assert node > base, f"node {base} degree > {P}"
            chunks.append((int(e_lo), int(starts[node]), int(base)))
        assert len(chunks) <= N_CH, f"core {core}: {len(chunks)} > {N_CH}"
        ed = np.zeros((P, N_CH, 6), np.float32)
        oh = np.zeros((P, N_CH, NT), np.float32)
        ebr = np.zeros((P, N_CH, 3), np.float32)
        for ci, (lo, hi, base) in enumerate(chunks):
            k = hi - lo
            ed[:k, ci, 0:3] = vec_all[lo:hi]
            ed[:k, ci, 3:6] = embS_all[lo:hi]
            rl = (recv_s[lo:hi] - base)
            oh[np.arange(k), ci, rl] = 1.0
            nxt = min(base + NT, n1)
            ebr[:, ci, :] = 0.0
        in_maps.append({
            "ed": np.ascontiguousarray(ed.reshape(P, N_CH * 6)),
            "oh": np.ascontiguousarray(oh.reshape(P, N_CH * NT)),
            "ebr": None,  # filled below
        })
        # ebr: partition p=(n,r) -> emb[node(ch,n)]
        eb = np.zeros((NT, N_CH, 3), np.float32)
        for ci, (lo, hi, base) in enumerate(chunks):
            hi_n = min(base + NT, n1)
            eb[:hi_n - base, ci, :] = emb[base:hi_n]
        ebp = np.repeat(eb, N_RBF, axis=0)  # [(16n,8r)=128, N_CH, 3]
        # careful: partition index = n*8 + r -> repeat each node row 8x
        in_maps[-1]["ebr"] = np.ascontiguousarray(ebp.reshape(P, N_CH * 3))
        chunk_meta.append(chunks)
    return in_maps, chunk_meta


_NC_CACHE = [None]
_IN_MAPS_CACHE = [None]


def kernel(**inputs):
    from concourse.bass_utils import run_bass_kernel_spmd
    nc, _ = _build_nc()
    in_maps, chunk_meta = _host_prep(inputs)
    _NC_CACHE[0] = nc
    _IN_MAPS_CACHE[0] = in_maps
    res = run_bass_kernel_spmd(nc, in_maps, core_ids=list(range(N_CORES)))
    out = np.zeros((N_NODES, N_RBF, NF, 9), np.float32)
    for core in range(N_CORES):
        slab = res.results[core]["out"].reshape(NT, N_RBF, N_CH, NF, 9)
        n0, n1 = core * PER, (core + 1) * PER
        chunks = chunk_meta[core]
        for ci, (lo, hi, base) in enumerate(chunks):
            nxt = chunks[ci + 1][2] if ci + 1 < len(chunks) else n1
            out[base:nxt] = slab[:nxt - base, :, ci]
    return out


# revision 14
# speedup vs baseline: 2.6547x; 1.0368x over previous
"""CACE GNN message-passing kernel for 8 trn2 NeuronCores (v2).

Node-parallel sharding (625 nodes/core), edges sorted by receiver and packed
into <=128-edge x <=16-node chunks. Key structure exploited: within a segment
all edges share the receiver node, so A[n,r,m,(cs,cr)] factorizes as
Atilde[n,r,m,cs] * emb_recv[n,cr]; the nu=2..4 symmetrization runs on 3
channels instead of 9 and the result is expanded by emb_recv^nu at the end.

Per core:
  1. edge factors: radial[8] (one wide ACT-Sin), unit vec,
     rhs = ordered-monomial basis (1,V,S9,T27) x emb_send  [128e, 120] f32,
     lhsT = onehot(node,16) x radial [128e, 128] f32 (on Pool engine),
  2. one f32 matmul per chunk -> PSUM, ACT copies -> Atilde slab (bf16),
  3. dense bf16 symmetrization (ordered index sets, strided tensor ops with
     <=3 free dims, tree adds) -> 11 features x 3 cs channels,
  4. f32 expansion by emb_recv^nu(f) -> [128p, ch, 11, 9] -> DMA out.
"""
import math
import functools
import numpy as np

# ---------------- problem constants ----------------------------------------
N_NODES, N_EDGES = 5000, 50000
N_RBF = 8
CUTOFF = 5.5
EPS = 1e-9
ZS = [1, 6, 7, 8]
N_CORES = 8
PER = N_NODES // N_CORES          # 625 nodes per core
NT = 16                           # nodes per chunk
N_CH = 53                         # chunks per core (padded; max measured 53)
P = 128                           # edges per chunk (partitions)
NQ = NT * N_RBF                   # 128 = lhsT free dim
MB = 40                           # ordered monomial planes: 1 + 3 + 9 + 27
NF = 11
SQ2C = math.sqrt(2.0 / CUTOFF)
CH = N_CH


# ---------------- device kernel build --------------------------------------
@functools.lru_cache(maxsize=2)
def _build_nc(debug=False):
    import concourse.bass as bass
    import concourse.bacc as bacc
    import concourse.mybir as mybir
    from concourse.tile import TileContext

    f32 = mybir.dt.float32
    bf16 = mybir.dt.bfloat16
    MUL = mybir.AluOpType.mult
    ADD = mybir.AluOpType.add
    SUB = mybir.AluOpType.subtract
    ACT = mybir.ActivationFunctionType
    AX = mybir.AxisListType.X

    nc = bacc.Bacc("TRN2", target_bir_lowering=False, debug=False,
                   num_devices=N_CORES)
    ed_d = nc.dram_tensor("ed", [P, CH * 6], f32, kind="ExternalInput")
    oh_d = nc.dram_tensor("oh", [P, CH * NT], f32, kind="ExternalInput")
    ebr_d = nc.dram_tensor("ebr", [P, CH * 3], f32, kind="ExternalInput")
    out_d = nc.dram_tensor("out", [P, CH * NF * 9], f32,
                           kind="ExternalOutput")
    dbg = {}
    if debug:
        for nm, w in [("radial", CH * N_RBF), ("unit", CH * 3),
                      ("rhs", CH * MB * 3), ("lhsT", CH * NQ)]:
            dbg[nm] = nc.dram_tensor("dbg_" + nm, [P, w], f32,
                                     kind="ExternalOutput")
        for nm, w in [("A", CH * MB * 3), ("Q", CH * 39 * 3),
                      ("Ft", CH * NF * 3)]:
            dbg[nm] = nc.dram_tensor("dbg_" + nm, [P, w], mybir.dt.bfloat16,
                                     kind="ExternalOutput")

    with TileContext(nc) as tc:
        with (
            tc.tile_pool(name="io", bufs=1) as io,
            tc.tile_pool(name="work", bufs=1) as wk,
            tc.tile_pool(name="psum", bufs=3, space="PSUM") as pp,
        ):
            ed = io.tile([P, CH * 6], f32)
            oh = io.tile([P, CH * NT], f32)
            ebr = io.tile([P, CH * 3], f32)
            nc.sync.dma_start(out=ed[:, :], in_=ed_d[:, :])
            nc.sync.dma_start(out=oh[:, :], in_=oh_d[:, :])
            nc.sync.dma_start(out=ebr[:, :], in_=ebr_d[:, :])

            edv = ed[:, :].rearrange("p (ch t) -> p ch t", t=6)
            vec = edv[:, :, 0:3]
            embS = edv[:, :, 3:6]

            # ---- geometry (f32) ----
            dsq = wk.tile([P, CH * 3], f32)
            dsqv = dsq[:, :].rearrange("p (ch t) -> p ch t", t=3)
            nc.vector.tensor_tensor(out=dsqv, in0=vec, in1=vec, op=MUL)
            l2 = wk.tile([P, CH], f32)
            nc.vector.tensor_reduce(out=l2[:, :], in_=dsqv, axis=AX, op=ADD)
            ln = wk.tile([P, CH], f32)
            nc.scalar.activation(out=ln[:, :], in_=l2[:, :], func=ACT.Sqrt)
            le = wk.tile([P, CH], f32)
            nc.vector.tensor_scalar_add(le[:, :], ln[:, :], EPS)
            rinv = wk.tile([P, CH], f32)
            nc.vector.reciprocal(out=rinv[:, :], in_=le[:, :])
            unit = wk.tile([P, CH * 3], f32)
            unitv = unit[:, :].rearrange("p (ch t) -> p ch t", t=3)
            nc.vector.tensor_tensor(
                out=unitv, in0=vec,
                in1=rinv[:, :].unsqueeze(2).to_broadcast([P, CH, 3]),
                op=MUL)

            # ---- radial (f32) ----
            # sin(n*theta) via Chebyshev recurrence (ACT Sin only valid near
            # [-pi, pi]): s_n = (2 - 4 sin^2(th/2)) s_{n-1} - s_{n-2}
            lc = wk.tile([P, CH], f32)
            nc.vector.tensor_scalar_min(lc[:, :], ln[:, :], CUTOFF)
            th = wk.tile([P, CH], f32)
            nc.vector.tensor_scalar_mul(th[:, :], lc[:, :], math.pi / CUTOFF)
            hh = wk.tile([P, CH], f32)
            nc.vector.tensor_scalar_mul(hh[:, :], lc[:, :],
                                        math.pi / (2.0 * CUTOFF))
            s2 = wk.tile([P, CH], f32)
            nc.scalar.activation(out=s2[:, :], in_=hh[:, :], func=ACT.Sin)
            s2q = wk.tile([P, CH], f32)
            nc.scalar.activation(out=s2q[:, :], in_=s2[:, :], func=ACT.Square)
            c2 = wk.tile([P, CH], f32)
            nc.vector.tensor_scalar(c2[:, :], s2q[:, :], -4.0, 2.0, MUL, ADD)
            sinr = wk.tile([P, CH * N_RBF], f32)
            sv = sinr[:, :].rearrange("p (ch r) -> p ch r", r=N_RBF)
            nc.scalar.activation(out=sv[:, :, 0], in_=th[:, :], func=ACT.Sin)
            nc.gpsimd.tensor_tensor(out=sv[:, :, 1], in0=c2[:, :],
                                    in1=sv[:, :, 0], op=MUL)
            for n in range(2, N_RBF):
                tmp_n = wk.tile([P, CH], f32, tag=f"cheb{n % 2}")
                nc.gpsimd.tensor_tensor(out=tmp_n[:, :], in0=c2[:, :],
                                        in1=sv[:, :, n - 1], op=MUL)
                nc.gpsimd.tensor_tensor(out=sv[:, :, n], in0=tmp_n[:, :],
                                        in1=sv[:, :, n - 2], op=SUB)
            uu = wk.tile([P, CH], f32)
            nc.vector.tensor_scalar_mul(uu[:, :], ln[:, :], 1.0 / CUTOFF)
            u2 = wk.tile([P, CH], f32)
            nc.vector.tensor_tensor(out=u2[:, :], in0=uu[:, :], in1=uu[:, :],
                                    op=MUL)
            u3 = wk.tile([P, CH], f32)
            nc.vector.tensor_tensor(out=u3[:, :], in0=u2[:, :], in1=uu[:, :],
                                    op=MUL)
            u6 = wk.tile([P, CH], f32)
            nc.vector.tensor_tensor(out=u6[:, :], in0=u3[:, :], in1=u3[:, :],
                                    op=MUL)
            t1 = wk.tile([P, CH], f32)
            nc.vector.tensor_scalar(t1[:, :], uu[:, :], -21.0, 48.0, MUL, ADD)
            t2 = wk.tile([P, CH], f32)
            nc.vector.tensor_tensor(out=t2[:, :], in0=t1[:, :], in1=uu[:, :],
                                    op=MUL)
            nc.vector.tensor_scalar_add(t2[:, :], t2[:, :], -28.0)
            fcv = wk.tile([P, CH], f32)
            nc.vector.tensor_tensor(out=fcv[:, :], in0=u6[:, :], in1=t2[:, :],
                                    op=MUL)
            nc.vector.tensor_scalar_add(fcv[:, :], fcv[:, :], 1.0)
            msk = wk.tile([P, CH], f32)
            nc.vector.tensor_scalar(msk[:, :], ln[:, :], CUTOFF, None,
                                    mybir.AluOpType.is_lt)
            nc.vector.tensor_tensor(out=fcv[:, :], in0=fcv[:, :],
                                    in1=msk[:, :], op=MUL)
            wfac = wk.tile([P, CH], f32)
            nc.vector.tensor_tensor(out=wfac[:, :], in0=fcv[:, :],
                                    in1=rinv[:, :], op=MUL)
            nc.vector.tensor_scalar_mul(wfac[:, :], wfac[:, :], SQ2C)
            radial = wk.tile([P, CH * N_RBF], f32)
            radv = radial[:, :].rearrange("p (ch r) -> p ch r", r=N_RBF)
            nc.vector.tensor_tensor(
                out=radv,
                in0=sinr[:, :].rearrange("p (ch r) -> p ch r", r=N_RBF),
                in1=wfac[:, :].unsqueeze(2).to_broadcast([P, CH, N_RBF]),
                op=MUL)

            # ---- lhsT = onehot x radial (f32) on Pool, split in two ----
            lhsT = wk.tile([P, CH * NQ], f32)
            lv = lhsT[:, :].rearrange("p (ch n r) -> p ch n r", n=NT, r=N_RBF)
            ohv = oh[:, :].rearrange("p (ch n) -> p ch n", n=NT)
            H1 = 27
            for lo, hi in ((0, H1), (H1, CH)):
                nc.gpsimd.tensor_tensor(
                    out=lv[:, lo:hi],
                    in0=ohv[:, lo:hi].unsqueeze(3).to_broadcast(
                        [P, hi - lo, NT, N_RBF]),
                    in1=radv[:, lo:hi].unsqueeze(2).to_broadcast(
                        [P, hi - lo, NT, N_RBF]),
                    op=MUL)

            # ---- rhs: ordered basis x embS, cascaded (f32) ----
            S9 = wk.tile([P, CH * 9], f32)
            s9v = S9[:, :].rearrange("p (ch a b) -> p ch a b", a=3, b=3)
            nc.vector.tensor_tensor(
                out=s9v,
                in0=unitv.unsqueeze(3).to_broadcast([P, CH, 3, 3]),
                in1=unitv.unsqueeze(2).to_broadcast([P, CH, 3, 3]),
                op=MUL)
            rhs = wk.tile([P, CH * MB * 3], f32)
            rv = rhs[:, :].rearrange("p (ch m c) -> p ch m c", m=MB, c=3)
            nc.scalar.copy(out=rv[:, :, 0, :], in_=embS)
            nc.vector.tensor_tensor(
                out=rv[:, :, 1:4, :],
                in0=unitv.unsqueeze(3).to_broadcast([P, CH, 3, 3]),
                in1=embS.unsqueeze(2).to_broadcast([P, CH, 3, 3]),
                op=MUL)
            ve9 = rv[:, :, 1:4, :].rearrange("p ch b c -> p ch (b c)")
            nc.vector.tensor_tensor(
                out=rv[:, :, 4:13, :].rearrange(
                    "p ch (a b) c -> p ch a (b c)", a=3),
                in0=unitv.unsqueeze(3).to_broadcast([P, CH, 3, 9]),
                in1=ve9.unsqueeze(2).to_broadcast([P, CH, 3, 9]),
                op=MUL)
            nc.vector.tensor_tensor(
                out=rv[:, :, 13:40, :].rearrange(
                    "p ch (ab cc) c -> p ch ab (cc c)", ab=9),
                in0=s9v.rearrange("p ch a b -> p ch (a b)")
                    .unsqueeze(3).to_broadcast([P, CH, 9, 9]),
                in1=ve9.unsqueeze(2).to_broadcast([P, CH, 9, 9]),
                op=MUL)

            # ---- matmuls (f32) + ACT psum->sbuf copies (bf16 A slab) ----
            A = wk.tile([P, CH * MB * 3], bf16)
            Av = A[:, :].rearrange("p (ch m c) -> p ch m c", m=MB, c=3)
            rflat = rhs[:, :].rearrange("p (ch f) -> p ch f", f=MB * 3)
            lflat = lhsT[:, :].rearrange("p (ch q) -> p ch q", q=NQ)
            Aflat = A[:, :].rearrange("p (ch f) -> p ch f", f=MB * 3)
            GW = 4
            n_grp = (CH + GW - 1) // GW
            for g in range(n_grp):
                c0 = g * GW
                c1 = min(CH, c0 + GW)
                pt = pp.tile([P, GW * MB * 3], f32)
                for ch in range(c0, c1):
                    k = ch - c0
                    nc.tensor.matmul(
                        out=pt[:, k * 120:(k + 1) * 120],
                        lhsT=lflat[:, ch, :], rhs=rflat[:, ch, :],
                        start=True, stop=True)
                nc.scalar.copy(
                    out=Aflat[:, c0:c1, :].rearrange("p ch f -> p (ch f)"),
                    in_=pt[:, :(c1 - c0) * 120])

            # ---- symmetrization (bf16, <=3 free dims per AP) ----
            # plane index helpers into A: m=0 const; V=1..3; S(a,b)=4+3a+b;
            # T(a,b,c)=13+9a+3b+c
            def APL(m0, m1=None):
                """contiguous A planes [m0, m1) as [p, ch, (m c)]"""
                m1 = m0 + 1 if m1 is None else m1
                return Av[:, :, m0:m1, :].rearrange("p ch m c -> p ch (m c)")

            with nc.allow_low_precision(reason="bf16 feature pipeline"):
                Q = wk.tile([P, CH * 39 * 3], bf16)
                a139 = Av[:, :, 1:40, :].rearrange("p ch m c -> p ch (m c)")
                qv3 = Q[:, :].rearrange("p (ch m c) -> p ch (m c)", m=39, c=3)
                nc.vector.tensor_tensor(out=qv3, in0=a139, in1=a139, op=MUL)
                Qv = Q[:, :].rearrange("p (ch m c) -> p ch m c", m=39, c=3)

                def QPL(m0, m1=None):
                    m1 = m0 + 1 if m1 is None else m1
                    return Qv[:, :, m0:m1, :].rearrange(
                        "p ch m c -> p ch (m c)")

                Ft = wk.tile([P, CH * NF * 3], bf16)
                Fv = Ft[:, :].rearrange("p (ch f c) -> p ch f c", f=NF, c=3)

                def FPL(f):
                    return Fv[:, :, f, :]

                def c3(dst, srcs, eng=None):
                    e = eng or nc.vector
                    e.tensor_tensor(out=dst, in0=srcs[0], in1=srcs[1], op=ADD)
                    e.tensor_tensor(out=dst, in0=dst, in1=srcs[2], op=ADD)

                # F0 = A plane 0
                nc.scalar.copy(out=FPL(0), in_=APL(0))
                # nu2_1 = sum_a V_a^2 (Q planes 0..2)
                c3(FPL(1), [QPL(i) for i in range(3)])
                # nu2_2 = sum S9^2 (Q planes 3..11)
                t9 = wk.tile([P, CH * 9], bf16)
                t9v = t9[:, :].rearrange("p (ch x) -> p ch x", x=9)
                c3(t9v, [QPL(3 + 3 * i, 6 + 3 * i) for i in range(3)])
                t3 = t9[:, :].rearrange("p (ch b c) -> p ch b c", b=3, c=3)
                c3(FPL(2), [t3[:, :, i, :] for i in range(3)])
                # nu2_3 = sum T27^2 (Q planes 12..38)
                t27 = wk.tile([P, CH * 27], bf16)
                t27v = t27[:, :].rearrange("p (ch x) -> p ch x", x=27)
                c3(t27v, [QPL(12 + 9 * i, 21 + 9 * i) for i in range(3)])
                t27b = t27[:, :].rearrange("p (ch b cc) -> p ch b cc", b=3,
                                           cc=9)
                c3(t9v, [t27b[:, :, i, :] for i in range(3)])
                c3(FPL(3), [t3[:, :, i, :] for i in range(3)])

                # u_a = sum_b S_ab V_b : per-a products [ch, (b c)=9]
                u9 = wk.tile([P, CH * 27], bf16)
                u9v = u9[:, :].rearrange("p (ch a bc) -> p ch a bc", a=3,
                                         bc=9)
                vpl = APL(1, 4)                      # [ch, 9]
                for a in range(3):
                    nc.vector.tensor_tensor(
                        out=u9v[:, :, a, :], in0=APL(4 + 3 * a, 7 + 3 * a),
                        in1=vpl, op=MUL)
                u_ = wk.tile([P, CH * 9], bf16)
                uv9 = u_[:, :].rearrange("p (ch x) -> p ch x", x=9)
                u9b = u9[:, :].rearrange("p (ch a b c) -> p ch a b c", a=3,
                                         b=3, c=3)
                uvb = u_[:, :].rearrange("p (ch a c) -> p ch a c", a=3, c=3)
                # contract b (middle): out [ch, 3a, 3c]
                nc.vector.tensor_tensor(out=uvb, in0=u9b[:, :, :, 0, :],
                                        in1=u9b[:, :, :, 1, :], op=ADD)
                nc.vector.tensor_tensor(out=uvb, in0=uvb,
                                        in1=u9b[:, :, :, 2, :], op=ADD)

                # tz_abc = T_abc * S_ab : per-cc products
                tz = wk.tile([P, CH * 81], bf16)
                tzv = tz[:, :].rearrange("p (ch ab cc c) -> p ch ab cc c",
                                         ab=9, cc=3, c=3)
                Tv = Av[:, :, 13:40, :].rearrange(
                    "p ch (ab cc) c -> p ch ab cc c", ab=9, cc=3)
                spl9 = Av[:, :, 4:13, :].rearrange(
                    "p ch (ab) c -> p ch ab c", ab=9)
                for cc in range(3):
                    nc.vector.tensor_tensor(
                        out=tzv[:, :, :, cc, :], in0=Tv[:, :, :, cc, :],
                        in1=spl9, op=MUL)
                # z_c = sum_ab tz: contract a then b
                tza = tz[:, :].rearrange("p (ch a rest) -> p ch a rest", a=3,
                                         rest=27)
                z9 = wk.tile([P, CH * 27], bf16)
                z9a = z9[:, :].rearrange("p (ch x) -> p ch x", x=27)
                c3(z9a, [tza[:, :, i, :] for i in range(3)])
                z9b = z9[:, :].rearrange("p (ch b y) -> p ch b y", b=3, y=9)
                z_ = wk.tile([P, CH * 9], bf16)
                zv9 = z_[:, :].rearrange("p (ch x) -> p ch x", x=9)
                c3(zv9, [z9b[:, :, i, :] for i in range(3)])
                zvb = z_[:, :].rearrange("p (ch cc c) -> p ch cc c", cc=3,
                                         c=3)

                # P2_bc = sum_a V_a T_abc : per-a products [ch, 9bc, 3c]
                pv = wk.tile([P, CH * 81], bf16)
                pvv = pv[:, :].rearrange("p (ch a x) -> p ch a x", a=3, x=27)
                pv4 = pv[:, :].rearrange("p (ch a bc c) -> p ch a bc c", a=3,
                                         bc=9, c=3)
                for a in range(3):
                    nc.vector.tensor_tensor(
                        out=pv4[:, :, a, :, :],
                        in0=Av[:, :, 13 + 9 * a:22 + 9 * a, :],
                        in1=Av[:, :, 1 + a, :].unsqueeze(2).to_broadcast(
                            [P, CH, 9, 3]),
                        op=MUL)
                p2 = wk.tile([P, CH * 27], bf16)
                p2v = p2[:, :].rearrange("p (ch x) -> p ch x", x=27)
                c3(p2v, [pvv[:, :, i, :] for i in range(3)])

                # S2_ab = sum_k S_ak S_kb : per-(a,k) products [ch, (b c)=9]
                ss = wk.tile([P, CH * 81], bf16)
                ss4 = ss[:, :].rearrange("p (ch ak b c) -> p ch ak b c",
                                         ak=9, b=3, c=3)
                for a in range(3):
                    for k in range(3):
                        # S_ak (bcast over b) * S[k, b]
                        nc.vector.tensor_tensor(
                            out=ss4[:, :, a * 3 + k, :, :],
                            in0=Av[:, :, 4 + 3 * a + k, :].unsqueeze(2)
                                .to_broadcast([P, CH, 3, 3]),
                            in1=Av[:, :, 4 + 3 * k:7 + 3 * k, :],
                            op=MUL)
                s2 = wk.tile([P, CH * 27], bf16)
                s2v = s2[:, :].rearrange("p (ch x) -> p ch x", x=27)
                ssk = ss[:, :].rearrange("p (ch a k bc) -> p ch a k bc", a=3,
                                         k=3, bc=9)
                # contract k (middle): [ch, 3a, 9bc]
                s2m = s2[:, :].rearrange("p (ch a bc) -> p ch a bc", a=3,
                                         bc=9)
                nc.vector.tensor_tensor(out=s2m, in0=ssk[:, :, :, 0, :],
                                        in1=ssk[:, :, :, 1, :], op=ADD)
                nc.vector.tensor_tensor(out=s2m, in0=s2m,
                                        in1=ssk[:, :, :, 2, :], op=ADD)

                # trS3 = sum_ab S2_ab S_ab
                w9 = wk.tile([P, CH * 27], bf16)
                w9v = w9[:, :].rearrange("p (ch x) -> p ch x", x=27)
                nc.vector.tensor_tensor(out=w9v, in0=s2v, in1=APL(4, 13),
                                        op=MUL)
                w9a = w9[:, :].rearrange("p (ch a y) -> p ch a y", a=3, y=9)
                c3(t9v, [w9a[:, :, i, :] for i in range(3)])
                c3(FPL(4), [t3[:, :, i, :] for i in range(3)])

                # M_cd (c<=d) = sum_ab T_abc T_abd : products [ch,9ab,6cd,3]
                mprod = wk.tile([P, CH * 9 * 6 * 3], bf16)
                mpv = mprod[:, :].rearrange("p (ch ab k c) -> p ch ab k c",
                                            ab=9, k=6, c=3)
                cdpairs = [(0, 0), (0, 1), (0, 2), (1, 1), (1, 2), (2, 2)]
                for ki, (c, dd) in enumerate(cdpairs):
                    nc.vector.tensor_tensor(
                        out=mpv[:, :, :, ki, :],
                        in0=Tv[:, :, :, c, :], in1=Tv[:, :, :, dd, :],
                        op=MUL)
                # contract ab: a then b
                mpa = mprod[:, :].rearrange("p (ch a rest) -> p ch a rest",
                                            a=3, rest=54)
                m54 = wk.tile([P, CH * 54], bf16)
                m54v = m54[:, :].rearrange("p (ch x) -> p ch x", x=54)
                c3(m54v, [mpa[:, :, i, :] for i in range(3)])
                m54b = m54[:, :].rearrange("p (ch b y) -> p ch b y", b=3,
                                           y=18)
                mm = wk.tile([P, CH * 18], bf16)
                mmv18 = mm[:, :].rearrange("p (ch x) -> p ch x", x=18)
                c3(mmv18, [m54b[:, :, i, :] for i in range(3)])
                mmv = mm[:, :].rearrange("p (ch k c) -> p ch k c", k=6, c=3)

                # nu3_2 = 2*sum_{c<=d} M_cd S_cd - sum_c M_cc S_cc
                q6 = wk.tile([P, CH * 18], bf16)
                q6v = q6[:, :].rearrange("p (ch k c) -> p ch k c", k=6, c=3)
                nc.vector.tensor_tensor(
                    out=q6v[:, :, 0:3, :].rearrange("p ch k c -> p ch (k c)"),
                    in0=mmv[:, :, 0:3, :].rearrange("p ch k c -> p ch (k c)"),
                    in1=APL(4, 7), op=MUL)
                nc.vector.tensor_tensor(
                    out=q6v[:, :, 3:5, :].rearrange("p ch k c -> p ch (k c)"),
                    in0=mmv[:, :, 3:5, :].rearrange("p ch k c -> p ch (k c)"),
                    in1=APL(8, 10), op=MUL)
                nc.vector.tensor_tensor(
                    out=q6v[:, :, 5, :], in0=mmv[:, :, 5, :], in1=APL(12),
                    op=MUL)
                sall = wk.tile([P, CH * 3], bf16)
                sallv = sall[:, :].rearrange("p (ch c) -> p ch c", c=3)
                t2s = wk.tile([P, CH * 6], bf16)
                t2sv = t2s[:, :].rearrange("p (ch x) -> p ch x", x=6)
                nc.vector.tensor_tensor(
                    out=t2sv,
                    in0=q6[:, :].rearrange("p (ch x) -> p ch x", x=18)[
                        :, :, 0:6],
                    in1=q6[:, :].rearrange("p (ch x) -> p ch x", x=18)[
                        :, :, 6:12], op=ADD)
                t2sk = t2s[:, :].rearrange("p (ch k c) -> p ch k c", k=2, c=3)
                nc.vector.tensor_tensor(out=sallv, in0=t2sk[:, :, 0, :],
                                        in1=t2sk[:, :, 1, :], op=ADD)
                nc.vector.tensor_tensor(out=sallv, in0=sallv,
                                        in1=q6v[:, :, 4, :], op=ADD)
                nc.vector.tensor_tensor(out=sallv, in0=sallv,
                                        in1=q6v[:, :, 5, :], op=ADD)
                sdia = wk.tile([P, CH * 3], bf16)
                sdiav = sdia[:, :].rearrange("p (ch c) -> p ch c", c=3)
                nc.vector.tensor_tensor(out=sdiav, in0=q6v[:, :, 0, :],
                                        in1=q6v[:, :, 3, :], op=ADD)
                nc.vector.tensor_tensor(out=sdiav, in0=sdiav,
                                        in1=q6v[:, :, 5, :], op=ADD)
                nc.vector.scalar_tensor_tensor(
                    out=FPL(5), in0=sallv, scalar=2.0, in1=sdiav,
                    op0=MUL, op1=SUB)

                # nu4 features
                uu3 = wk.tile([P, CH * 9], bf16)
                uu3v = uu3[:, :].rearrange("p (ch x) -> p ch x", x=9)
                uu3b = uu3[:, :].rearrange("p (ch a c) -> p ch a c", a=3, c=3)
                nc.vector.tensor_tensor(out=uu3v, in0=uv9, in1=uv9, op=MUL)
                c3(FPL(6), [uu3b[:, :, i, :] for i in range(3)])
                nc.vector.tensor_tensor(out=uu3v, in0=uv9, in1=zv9, op=MUL)
                c3(FPL(7), [uu3b[:, :, i, :] for i in range(3)])
                nc.scalar.copy(out=FPL(9), in_=FPL(7))
                nc.vector.tensor_tensor(out=uu3v, in0=zv9, in1=zv9, op=MUL)
                c3(FPL(10), [uu3b[:, :, i, :] for i in range(3)])
                pp9 = wk.tile([P, CH * 27], bf16)
                pp9v = pp9[:, :].rearrange("p (ch x) -> p ch x", x=27)
                nc.vector.tensor_tensor(out=pp9v, in0=p2v, in1=p2v, op=MUL)
                pp9b = pp9[:, :].rearrange("p (ch b cc) -> p ch b cc", b=3,
                                           cc=9)
                c3(t9v, [pp9b[:, :, i, :] for i in range(3)])
                c3(FPL(8), [t3[:, :, i, :] for i in range(3)])

            # ---- expansion by emb_recv^nu(f) (f32) ----
            ebv = ebr[:, :].rearrange("p (ch c) -> p ch c", c=3)
            e2 = wk.tile([P, CH * 3], f32)
            e2v = e2[:, :].rearrange("p (ch c) -> p ch c", c=3)
            nc.vector.tensor_tensor(out=e2v, in0=ebv, in1=ebv, op=MUL)
            e3 = wk.tile([P, CH * 3], f32)
            e3v = e3[:, :].rearrange("p (ch c) -> p ch c", c=3)
            nc.vector.tensor_tensor(out=e3v, in0=e2v, in1=ebv, op=MUL)
            e4 = wk.tile([P, CH * 3], f32)
            e4v = e4[:, :].rearrange("p (ch c) -> p ch c", c=3)
            nc.vector.tensor_tensor(out=e4v, in0=e2v, in1=e2v, op=MUL)

            outt = wk.tile([P, CH * NF * 9], f32)
            ov = outt[:, :].rearrange("p (ch f cs cr) -> p ch f cs cr", f=NF,
                                      cs=3, cr=3)
            nu_of_f = [1, 2, 2, 2, 3, 3, 4, 4, 4, 4, 4]
            epows = {1: ebv, 2: e2v, 3: e3v, 4: e4v}
            for f in range(NF):
                nc.vector.tensor_tensor(
                    out=ov[:, :, f, :, :],
                    in0=Fv[:, :, f, :].unsqueeze(3).to_broadcast(
                        [P, CH, 3, 3]),
                    in1=epows[nu_of_f[f]].unsqueeze(2).to_broadcast(
                        [P, CH, 3, 3]),
                    op=MUL)
            nc.sync.dma_start(out=out_d[:, :], in_=outt[:, :])
            if debug:
                for nm, src in [("radial", radial), ("unit", unit),
                                ("rhs", rhs), ("lhsT", lhsT)]:
                    nc.sync.dma_start(out=dbg[nm][:, :], in_=src[:, :])
                for nm, src in [("A", A), ("Q", Q), ("Ft", Ft)]:
                    nc.sync.dma_start(out=dbg[nm][:, :], in_=src[:, :])
    nc.compile()
    return nc, None


# ---------------- host side -------------------------------------------------
def _host_prep(inputs):
    pos = np.ascontiguousarray(inputs['positions'], np.float32)
    W = np.asarray(inputs['W_embed'], np.float32)
    an = np.asarray(inputs['atomic_numbers'])
    ei = np.asarray(inputs['edge_index'])
    shifts = np.asarray(inputs.get('shifts'), np.float32)
    zs = np.asarray(ZS, an.dtype)
    onehot = (an[:, None] == zs[None, :]).astype(np.float32)
    emb = onehot @ W
    send, recv = ei[0], ei[1]
    order = np.argsort(recv, kind='stable')
    send_s, recv_s = send[order], recv[order]
    vec_all = pos[recv_s] - pos[send_s] + shifts[order]
    embS_all = emb[send_s]
    counts = np.bincount(recv_s, minlength=N_NODES)
    starts = np.concatenate([[0], np.cumsum(counts)])
    in_maps = []
    chunk_meta = []
    for core in range(N_CORES):
        n0, n1 = core * PER, (core + 1) * PER
        chunks = []
        node = n0
        while node < n1:
            base = node
            e_lo = starts[node]
            while (node < n1 and node - base < NT
                   and starts[node + 1] - e_lo <= P):
                node += 1
            assert node > base, f"node {base} degree > {P}"
            chunks.append((int(e_lo), int(starts[node]), int(base)))
        assert len(chunks) <= N_CH, f"core {core}: {len(chunks)} > {N_CH}"
        ed = np.zeros((P, N_CH, 6), np.float32)
        oh = np.zeros((P, N_CH, NT), np.float32)
        eb = np.zeros((NT, N_CH, 3), np.float32)
        for ci, (lo, hi, base) in enumerate(chunks):
            k = hi - lo
            ed[:k, ci, 0:3] = vec_all[lo:hi]
            ed[:k, ci, 3:6] = embS_all[lo:hi]
            rl = recv_s[lo:hi] - base
            oh[np.arange(k), ci, rl] = 1.0
            hi_n = min(base + NT, n1)
            eb[:hi_n - base, ci, :] = emb[base:hi_n]
        ebp = np.repeat(eb, N_RBF, axis=0)   # partition p = n*8 + r
        in_maps.append({
            "ed": np.ascontiguousarray(ed.reshape(P, N_CH * 6)),
            "oh": np.ascontiguousarray(oh.reshape(P, N_CH * NT)),
            "ebr": np.ascontiguousarray(ebp.reshape(P, N_CH * 3)),
        })
        chunk_meta.append(chunks)
    return in_maps, chunk_meta


_NC_CACHE = [None]
_IN_MAPS_CACHE = [None]


def kernel(**inputs):
    from concourse.bass_utils import run_bass_kernel_spmd
    nc, _ = _build_nc()
    in_maps, chunk_meta = _host_prep(inputs)
    _NC_CACHE[0] = nc
    _IN_MAPS_CACHE[0] = in_maps
    res = run_bass_kernel_spmd(nc, in_maps, core_ids=list(range(N_CORES)))
    out = np.zeros((N_NODES, N_RBF, NF, 9), np.float32)
    for core in range(N_CORES):
        slab = res.results[core]["out"].reshape(NT, N_RBF, N_CH, NF, 9)
        n0, n1 = core * PER, (core + 1) * PER
        chunks = chunk_meta[core]
        for ci, (lo, hi, base) in enumerate(chunks):
            nxt = chunks[ci + 1][2] if ci + 1 < len(chunks) else n1
            out[base:nxt] = slab[:nxt - base, :, ci]
    return out
